# revision 1
# baseline (speedup 1.0000x reference)
"""Trainium2 Bass kernel for a 2-layer GCN encoder (PyG GCNConv semantics).

  out = A_hat @ (relu(A_hat @ (x @ W1) + b1) @ W2) + b2
  A_hat = D^-1/2 (A + I) D^-1/2,  deg computed on dst (col) with self loops.

Strategy (8 NeuronCores, SPMD, node sharding):
  - Host (index-only prep): build self-loop edge list, degrees, dis=1/sqrt(deg),
    bin edges by (dst shard, dst-tile chunk, src window, dst tile), sort by src,
    pad groups to uniform (across cores) multiples of 128, precompute wrapped
    int16 gather indices and per-edge (dst-in-tile, dis[dst]) columns.
    Also pre-transpose/shard x (layout-only) and pad weights.
  - Phase A: each core computes p1 = dis * (x_shard @ W1) with PE matmuls,
    AllGather -> full p1 table [NP, H] in every core's DRAM.
  - L1 propagate: per dst-tile, dma_gather rows of p1 (512B/edge) by int16
    indices (two windows: rows [0,32768) and [32768,NP) keep idx in int16
    range), build S[e,d] = (dstloc_e==d)*dis_dst_e with one fused DVE
    tensor_scalar per 128-edge tile, reduce via PE: psum[f,d] += msg^T... i.e.
    matmul(lhsT=msg[e,f], rhs=S[e,d]) accumulated over the tile's edge tiles.
    Evacuate with ACT: h1T[f,d] = relu(psum + b1).
  - L2 transform: p2[d,64] = matmul(lhsT=h1T[f,d], rhs=W2[f,64]); AllGather p2.
  - L2 propagate: gather p2 rows (256B/edge), same S, out[d,64] =
    matmul(lhsT=S[e,d], rhs=msg2[e,64]) accumulated; add b2 (free-axis
    broadcast via pre-replicated b2 tile); DMA out.
"""

import math
import os
import sys
import types

import numpy as np

import concourse.bacc as bacc
import concourse.bass as bass
import concourse.mybir as mybir
import concourse.tile as tile
from concourse import bass_utils


def _install_ntff_hook():
    """Bridge the missing antenv.axon_hooks so trace=True works under axon."""
    try:
        if "antenv.axon_hooks" in sys.modules:
            return
        import trn_agent_boot.trn_boot as tb

        hook = tb._ntff_profile_via_ctypes("/opt/axon/libaxon_pjrt.so")
        mod = types.ModuleType("antenv.axon_hooks")
        mod.get_axon_ntff_profile_hook = lambda: hook
        mod.set_axon_ntff_profile_hook = lambda h: None
        sys.modules["antenv.axon_hooks"] = mod
        import antenv

        antenv.axon_hooks = mod
        bass_utils.upload_artifacts = lambda tmpdir: tmpdir
    except Exception:
        pass

P = 128

FULL_CFG = dict(N=50000, E=800000, IN=500, H=128, OUT=64, NC=8, CHUNK=2)

LAST_RESULTS = None  # test harness reads exec_time_ns from here


# --------------------------------------------------------------------------
# Host-side preprocessing (index manipulation + input layout only)
# --------------------------------------------------------------------------


def _ceil_to(a, m):
    return (a + m - 1) // m * m


def _wrap16(idx):
    """[G] int16 -> [128, G//16]: edge j at partition j%16 slot j//16, x8 replicated."""
    g = idx.shape[0]
    w = idx.reshape(g // 16, 16).T
    return np.ascontiguousarray(np.tile(w, (8, 1)))


def _wrap128(v):
    """[G] -> [128, G//128]: edge j at partition j%128 slot j//128."""
    g = v.shape[0]
    return np.ascontiguousarray(v.reshape(g // P, P).T)


def _plan_and_prep(x, edge_index, W1, b1, W2, b2, cfg):
    N, E, IN, H, OUT, NC = (
        cfg["N"], cfg["E"], cfg["IN"], cfg["H"], cfg["OUT"], cfg["NC"],
    )
    CHUNK = cfg["CHUNK"]
    NL = N // NC                      # real nodes per core
    NLP = _ceil_to(NL, P)             # padded nodes per core
    TPC = NLP // P                    # dst tiles per core
    NP = NLP * NC                     # padded global nodes
    INP = _ceil_to(IN, P)             # padded input feature dim
    KC = INP // P                     # k chunks for x @ W1
    LOW = min(32768, NP)              # lo window rows

    # ---- graph with self loops, degrees, dis ----
    src = np.concatenate([edge_index[0].astype(np.int64), np.arange(N, dtype=np.int64)])
    dst = np.concatenate([edge_index[1].astype(np.int64), np.arange(N, dtype=np.int64)])
    deg = np.bincount(dst, minlength=N).astype(np.float32)
    dis = (1.0 / np.sqrt(np.maximum(deg, 1.0))).astype(np.float32)
    dis = np.where(deg > 0, dis, 0.0).astype(np.float32)  # deg>=1 always, keep parity

    core = dst // NL                  # owning core of dst
    dloc = dst % NL                   # local dst id
    t_of = dloc // P                  # dst tile within core
    dint = (dloc % P).astype(np.float32)  # dst id within tile
    srcp = (src // NL) * NLP + (src % NL)  # padded global src id
    half = (srcp >= LOW).astype(np.int64)  # 0 = lo window, 1 = hi window
    disd = dis[dst]                   # dis of dst node

    # ---- group counts and uniform tile counts ----
    # group id = ((core * TPC) + t) * 2 + half
    gid = (core * TPC + t_of) * 2 + half
    cnt = np.bincount(gid, minlength=NC * TPC * 2).reshape(NC, TPC, 2)
    tiles_th = np.ceil(cnt.max(axis=0) / P).astype(np.int64)  # [TPC, 2]

    # ---- chunk layout: for chunk -> for half -> for t in chunk ----
    nchunks = math.ceil(TPC / CHUNK)
    chunk_tiles = [list(range(c * CHUNK, min((c + 1) * CHUNK, TPC))) for c in range(nchunks)]
    base_tile = np.zeros((TPC, 2), np.int64)  # edge-tile offset of group (t, half)
    gathers = []  # per chunk: dict(half -> (base_tile, n_tiles))
    mm_order = []  # per dst tile t: list of global edge-tile indices (lo tiles then hi)
    pos = 0
    tile_pos_of_t = [[[], []] for _ in range(TPC)]
    for tlist in chunk_tiles:
        ginfo = {}
        for h in (0, 1):
            b = pos
            for t in tlist:
                base_tile[t, h] = pos
                tile_pos_of_t[t][h] = list(range(pos, pos + tiles_th[t, h]))
                pos += tiles_th[t, h]
            ginfo[h] = (b, pos - b)
        gathers.append(ginfo)
    total_tiles = pos
    GP = total_tiles * P
    for t in range(TPC):
        mm_order.append(tile_pos_of_t[t][0] + tile_pos_of_t[t][1])

    # ---- per-core padded edge arrays ----
    # slot of edge = base_tile[t,half]*128 + rank within (core,t,half) group
    order = np.lexsort((srcp, half, t_of, core))
    gid_sorted = gid[order]
    # rank within group
    first = np.ones(len(order), bool)
    first[1:] = gid_sorted[1:] != gid_sorted[:-1]
    group_start = np.where(first)[0]
    start_of = np.zeros(NC * TPC * 2, np.int64)
    start_of[gid_sorted[group_start]] = group_start
    rank = np.arange(len(order)) - start_of[gid_sorted]
    slot = base_tile[t_of[order], half[order]] * P + rank

    idx16 = np.zeros((NC, GP), np.int16)
    dstloc = np.full((NC, GP), -1.0, np.float32)
    c_sorted = core[order]
    win_idx = srcp[order] - half[order] * LOW
    assert win_idx.max() < 32768
    idx16[c_sorted, slot] = win_idx.astype(np.int16)
    dstloc[c_sorted, slot] = dint[order]

    # ---- per-core dense inputs ----
    x = np.asarray(x, np.float32)
    W1 = np.asarray(W1, np.float32)
    W2 = np.asarray(W2, np.float32)
    b1 = np.asarray(b1, np.float32)
    b2 = np.asarray(b2, np.float32)

    W1p = np.zeros((INP, H), np.float32)
    W1p[:IN] = W1
    iota = np.ascontiguousarray(np.tile(np.arange(P, dtype=np.float32), (P, 1)))
    b2rep = np.ascontiguousarray(np.tile(b2.reshape(1, OUT), (P, 1)))

    in_maps = []
    for c in range(NC):
        rows = slice(c * NL, (c + 1) * NL)
        xT = np.zeros((INP, NLP), np.float32)
        xT[:IN, :NL] = x[rows].T
        disl = np.zeros(NLP, np.float32)
        disl[:NL] = dis[rows]
        in_maps.append(
            {
                "xT": xT,
                "w1": W1p,
                "w2": W2,
                "b1": b1.reshape(H, 1).copy(),
                "b2rep": b2rep,
                "iota": iota,
                "dis_local": _wrap128(disl),
                "dis_rep": np.ascontiguousarray(np.tile(disl.reshape(1, NLP), (P, 1))),
                "idx": _wrap16(idx16[c]),
                "dstloc": _wrap128(dstloc[c]),
            }
        )

    plan = dict(
        cfg=cfg, NL=NL, NLP=NLP, TPC=TPC, NP=NP, INP=INP, KC=KC, LOW=LOW,
        GP=GP, total_tiles=total_tiles, gathers=gathers, mm_order=mm_order,
        chunk_tiles=chunk_tiles,
    )
    return plan, in_maps


# --------------------------------------------------------------------------
# Device program
# --------------------------------------------------------------------------


def _build_program(plan):
    cfg = plan["cfg"]
    N, IN, H, OUT, NC = cfg["N"], cfg["IN"], cfg["H"], cfg["OUT"], cfg["NC"]
    NLP, TPC, NP, INP, KC, LOW, GP = (
        plan["NLP"], plan["TPC"], plan["NP"], plan["INP"], plan["KC"],
        plan["LOW"], plan["GP"],
    )
    gathers, mm_order = plan["gathers"], plan["mm_order"]
    f32 = mybir.dt.float32
    HIW = NP - LOW

    nc = bacc.Bacc("TRN2", target_bir_lowering=False, debug=False, num_swdge_queues=4)

    xT_d = nc.dram_tensor("xT", [INP, NLP], f32, kind="ExternalInput")
    w1_d = nc.dram_tensor("w1", [INP, H], f32, kind="ExternalInput")
    w2_d = nc.dram_tensor("w2", [H, OUT], f32, kind="ExternalInput")
    b1_d = nc.dram_tensor("b1", [H, 1], f32, kind="ExternalInput")
    b2rep_d = nc.dram_tensor("b2rep", [P, OUT], f32, kind="ExternalInput")
    iota_d = nc.dram_tensor("iota", [P, P], f32, kind="ExternalInput")
    disl_d = nc.dram_tensor("dis_local", [P, TPC], f32, kind="ExternalInput")
    disrep_d = nc.dram_tensor("dis_rep", [P, NLP], f32, kind="ExternalInput")
    idx_d = nc.dram_tensor("idx", [P, GP // 16], mybir.dt.int16, kind="ExternalInput")
    dstloc_d = nc.dram_tensor("dstloc", [P, GP // P], f32, kind="ExternalInput")

    p1_local = nc.dram_tensor("p1_local", [NLP, H], f32)
    p1_full = nc.dram_tensor("p1_full", [NP, H], f32, addr_space="Shared")
    p2_local = nc.dram_tensor("p2_local", [NLP, OUT], f32)
    p2_full = nc.dram_tensor("p2_full", [NP, OUT], f32, addr_space="Shared")
    out_d = nc.dram_tensor("out_local", [NLP, OUT], f32, kind="ExternalOutput")

    with tile.TileContext(nc) as tc:
        # ================= Phase A: p1 = dis * (x @ W1), AllGather ========
        with (
            tc.tile_pool(name="xa", bufs=1) as xa_pool,
            tc.tile_pool(name="wa", bufs=1) as wa_pool,
            tc.tile_pool(name="pa", bufs=4, space="PSUM") as pa_psum,
            tc.tile_pool(name="sa", bufs=1) as sa_pool,
        ):
            w1_sb = wa_pool.tile([P, KC, H], f32)
            nc.sync.dma_start(w1_sb[:], w1_d.rearrange("(k p) h -> p k h", p=P))
            disl_sb = wa_pool.tile([P, TPC], f32)
            nc.sync.dma_start(disl_sb[:], disl_d[:])

            xk = xa_pool.tile([P, KC, NLP], f32)
            nc.sync.dma_start(xk[:], xT_d.rearrange("(k p) n -> p k n", p=P))

            p1_stage = sa_pool.tile([P, TPC, H], f32)
            for t in range(TPC):
                ps = pa_psum.tile([P, H], f32, space="PSUM")
                for k in range(KC):
                    nc.tensor.matmul(
                        out=ps[:],
                        lhsT=xk[:, k, t * P : (t + 1) * P],
                        rhs=w1_sb[:, k, :],
                        start=(k == 0),
                        stop=(k == KC - 1),
                    )
                nc.scalar.activation(
                    out=p1_stage[:, t, :],
                    in_=ps[:],
                    func=mybir.ActivationFunctionType.Copy,
                    bias=0.0,
                    scale=disl_sb[:, t : t + 1],
                )
            nc.sync.dma_start(
                p1_local.rearrange("(t p) h -> p t h", p=P), p1_stage[:]
            )
            nc.gpsimd.collective_compute(
                "AllGather",
                mybir.AluOpType.bypass,
                replica_groups=[list(range(NC))],
                ins=[p1_local[:]],
                outs=[p1_full[:]],
            )

        # ================= Phase B/C/D =====================================
        max_chunk_tiles = max(g[0][1] + g[1][1] for g in gathers)
        with (
            tc.tile_pool(name="const", bufs=1) as const_pool,
            tc.tile_pool(name="h1t", bufs=1) as h1_pool,
            tc.tile_pool(name="msg", bufs=2) as msg_pool,
            tc.tile_pool(name="s", bufs=5) as s_pool,
            tc.tile_pool(name="pb", bufs=2, space="PSUM") as pb_psum,
            tc.tile_pool(name="stage", bufs=1) as stage_pool,
        ):
            iota_sb = const_pool.tile([P, P], f32)
            nc.sync.dma_start(iota_sb[:], iota_d[:])
            b1_sb = const_pool.tile([H, 1], f32)
            nc.sync.dma_start(b1_sb[:], b1_d[:])
            b2rep_sb = const_pool.tile([P, OUT], f32)
            nc.sync.dma_start(b2rep_sb[:], b2rep_d[:])
            disl_sb2 = const_pool.tile([P, TPC], f32)
            nc.sync.dma_start(disl_sb2[:], disl_d[:])
            w2_sb = const_pool.tile([H, OUT], f32)
            nc.sync.dma_start(w2_sb[:], w2_d[:])
            idx_sb = const_pool.tile([P, GP // 16], mybir.dt.int16)
            nc.sync.dma_start(idx_sb[:], idx_d[:])
            dstloc_sb = const_pool.tile([P, GP // P], f32)
            nc.sync.dma_start(dstloc_sb[:], dstloc_d[:])
            disrep_sb = const_pool.tile([P, NLP], f32)
            nc.sync.dma_start(disrep_sb[:], disrep_d[:])

            h1T = h1_pool.tile([H, TPC, P], f32)

            BS = 16  # edge tiles per one-hot build block
            qctr = [0]

            def propagate(table_d, feat, out_cb):
                """Gather-reduce over all dst tiles."""
                for ci, tlist in enumerate(plan["chunk_tiles"]):
                    g = gathers[ci]
                    nt_lo, nt_hi = g[0][1], g[1][1]
                    ntot = nt_lo + nt_hi
                    if ntot == 0:
                        continue
                    msg = msg_pool.tile([P, max_chunk_tiles, feat], f32, tag="msg")
                    cbase = g[0][0]  # first edge-tile of this chunk
                    for h, off in ((0, 0), (1, nt_lo)):
                        nt = g[h][1]
                        if not nt:
                            continue
                        nidx = nt * P
                        lohi = (0, LOW) if h == 0 else (LOW, NP)
                        nc.gpsimd.dma_gather(
                            msg[:, off : off + nt, :],
                            table_d[lohi[0] : lohi[1], :],
                            idx_sb[:, g[h][0] * 8 : (g[h][0] + nt) * 8],
                            nidx,
                            nidx,
                            feat,
                            single_packet=False,
                            queue_num=qctr[0] % 4,
                        )
                        qctr[0] += 1

                    # blocked one-hot builds: S[e, d] = (dstloc_e == d)
                    s_blocks = []
                    for b0 in range(0, ntot, BS):
                        bn = min(BS, ntot - b0)
                        s_blk = s_pool.tile([P, BS * P], f32, tag="s")
                        dst_b = (
                            dstloc_sb[:, cbase + b0 : cbase + b0 + bn]
                            .unsqueeze(2)
                            .broadcast_to([P, bn, P])
                        )
                        io_b = iota_sb[:].unsqueeze(1).broadcast_to([P, bn, P])
                        nc.vector.tensor_tensor(
                            out=s_blk[:, : bn * P].rearrange(
                                "p (m f) -> p m f", m=bn
                            ),
                            in0=io_b,
                            in1=dst_b,
                            op=mybir.AluOpType.is_equal,
                        )
                        s_blocks.append(s_blk)

                    for t in tlist:
                        tiles = mm_order[t]
                        if not tiles:
                            continue
                        ps = pb_psum.tile([P, P], f32, space="PSUM", tag="ps")
                        for j, gt in enumerate(tiles):
                            k = gt - cbase
                            s_t = s_blocks[k // BS][:, (k % BS) * P : (k % BS + 1) * P]
                            m = msg[:, k, :]
                            out_cb(t, ps, s_t, m, j == 0, j == len(tiles) - 1)

            # ---- L1: psum[f, d] += msg^T(e,f) x S(e,d) -> h1T = relu(+b1)
            def l1_mm(t, ps, s_t, m, first, last):
                nc.tensor.matmul(
                    out=ps[:, :], lhsT=m, rhs=s_t, start=first, stop=last
                )
                if last:
                    tmp = s_pool.tile([P, P], f32, tag="ev1")
                    nc.vector.tensor_tensor(
                        out=tmp[:],
                        in0=ps[:, :],
                        in1=disrep_sb[:, t * P : (t + 1) * P],
                        op=mybir.AluOpType.mult,
                    )
                    nc.scalar.activation(
                        out=h1T[:, t, :],
                        in_=tmp[:],
                        func=mybir.ActivationFunctionType.Relu,
                        bias=b1_sb[:],
                        scale=1.0,
                    )

            propagate(p1_full, H, l1_mm)

            # ---- L2 transform: p2[d, OUT] = h1T(f,d)^T @ W2(f,OUT)
            p2_stage = stage_pool.tile([P, TPC, OUT], f32, tag="p2s")
            for t in range(TPC):
                ps = pb_psum.tile([P, OUT], f32, space="PSUM", tag="p2p")
                nc.tensor.matmul(
                    out=ps[:], lhsT=h1T[:, t, :], rhs=w2_sb[:], start=True, stop=True
                )
                nc.scalar.activation(
                    out=p2_stage[:, t, :],
                    in_=ps[:],
                    func=mybir.ActivationFunctionType.Copy,
                    bias=0.0,
                    scale=disl_sb2[:, t : t + 1],
                )
            nc.sync.dma_start(
                p2_local.rearrange("(t p) h -> p t h", p=P), p2_stage[:]
            )
            nc.gpsimd.collective_compute(
                "AllGather",
                mybir.AluOpType.bypass,
                replica_groups=[list(range(NC))],
                ins=[p2_local[:]],
                outs=[p2_full[:]],
            )

            # ---- L2 propagate: psum[d, OUT] += S(e,d)^T @ msg2(e,OUT)
            out_stage = stage_pool.tile([P, TPC, OUT], f32, tag="outs")

            def l2_mm(t, ps, s_t, m, first, last):
                nc.tensor.matmul(
                    out=ps[:, :OUT], lhsT=s_t, rhs=m, start=first, stop=last
                )
                if last:
                    tmp = s_pool.tile([P, OUT], f32, tag="ev2")
                    nc.vector.tensor_scalar(
                        out=tmp[:],
                        in0=ps[:, :OUT],
                        scalar1=disl_sb2[:, t : t + 1],
                        scalar2=None,
                        op0=mybir.AluOpType.mult,
                    )
                    nc.vector.tensor_tensor(
                        out=out_stage[:, t, :],
                        in0=tmp[:],
                        in1=b2rep_sb[:],
                        op=mybir.AluOpType.add,
                    )

            propagate(p2_full, OUT, l2_mm)

            nc.sync.dma_start(
                out_d.rearrange("(t p) h -> p t h", p=P), out_stage[:]
            )

    nc.compile()
    return nc


# --------------------------------------------------------------------------
# Entry point
# --------------------------------------------------------------------------


def _run(inputs, cfg=None, trace=False):
    global LAST_RESULTS
    cfg = dict(FULL_CFG if cfg is None else cfg)
    plan, in_maps = _plan_and_prep(
        inputs["x"], inputs["edge_index"], inputs["W1"], inputs["b1"],
        inputs["W2"], inputs["b2"], cfg,
    )
    nc = _build_program(plan)
    if trace:
        _install_ntff_hook()
    res = bass_utils.run_bass_kernel_spmd(
        nc, in_maps, core_ids=list(range(cfg["NC"])), trace=trace
    )
    LAST_RESULTS = res
    NL = plan["NL"]
    out = np.concatenate(
        [res.results[c]["out_local"][:NL] for c in range(cfg["NC"])], axis=0
    )
    return out.astype(np.float32)


def kernel(**inputs):
    return _run(inputs, trace=bool(os.environ.get("GCN_TRACE")))



# revision 2
# speedup vs baseline: 1.4859x; 1.4859x over previous
"""Trainium2 Bass kernel for a 2-layer GCN encoder (PyG GCNConv semantics).

  out = A_hat @ (relu(A_hat @ (x @ W1) + b1) @ W2) + b2
  A_hat = D^-1/2 (A + I) D^-1/2,  deg computed on dst (col) with self loops.

Strategy (8 NeuronCores, SPMD, node sharding), v2:
  - Self-loop edges are NOT materialized: the diagonal contribution is added
    per dst tile with one identity matmul against the locally available
    p1/p2 tile (PE-only, no gather traffic).
  - All tables/matmul operands are bf16 (f32 PSUM accumulate): p1_full
    [NP,128] bf16, p2_full [NP,128] bf16 (64 real cols + pad so gather rows
    stay 256B). Halves HBM/collective traffic and doubles PE rate.
  - Phase A: p1 = dis * (x @ W1) per shard, AllGather -> p1_full.
  - Propagate (both layers): per dst-tile chunk, dma_gather rows by int16
    indices (lo/hi windows keep idx in int16 range), build one-hot
    S[e,d] = (dstloc_e == d) in bf16 on DVE, reduce via PE matmuls into
    PSUM, add diag via identity matmul, evacuate with DVE+ACT.
  - L2 transform: p2 = dis * (h1T^T @ W2); AllGather p2 (padded rows).
  - Host does index prep + layout/dtype conversion only.
"""

import math
import os
import sys
import types

import numpy as np
import ml_dtypes

import concourse.bacc as bacc
import concourse.bass as bass
import concourse.mybir as mybir
import concourse.tile as tile
from concourse import bass_utils


def _install_ntff_hook():
    """Bridge the missing antenv.axon_hooks so trace=True works under axon."""
    try:
        if "antenv.axon_hooks" in sys.modules:
            return
        import trn_agent_boot.trn_boot as tb

        hook = tb._ntff_profile_via_ctypes("/opt/axon/libaxon_pjrt.so")
        mod = types.ModuleType("antenv.axon_hooks")
        mod.get_axon_ntff_profile_hook = lambda: hook
        mod.set_axon_ntff_profile_hook = lambda h: None
        sys.modules["antenv.axon_hooks"] = mod
        import antenv

        antenv.axon_hooks = mod
        bass_utils.upload_artifacts = lambda tmpdir: tmpdir
    except Exception:
        pass

P = 128
BF16 = ml_dtypes.bfloat16

FULL_CFG = dict(N=50000, E=800000, IN=500, H=128, OUT=64, NC=8, CHUNK=2)

LAST_RESULTS = None  # test harness reads exec_time_ns from here


# --------------------------------------------------------------------------
# Host-side preprocessing (index manipulation + input layout only)
# --------------------------------------------------------------------------


def _ceil_to(a, m):
    return (a + m - 1) // m * m


def _wrap16(idx):
    """[G] int16 -> [128, G//16]: edge j at partition j%16 slot j//16, x8 replicated."""
    g = idx.shape[0]
    w = idx.reshape(g // 16, 16).T
    return np.ascontiguousarray(np.tile(w, (8, 1)))


def _wrap128(v):
    """[G] -> [128, G//128]: edge j at partition j%128 slot j//128."""
    g = v.shape[0]
    return np.ascontiguousarray(v.reshape(g // P, P).T)


def _plan_and_prep(x, edge_index, W1, b1, W2, b2, cfg):
    N, E, IN, H, OUT, NC = (
        cfg["N"], cfg["E"], cfg["IN"], cfg["H"], cfg["OUT"], cfg["NC"],
    )
    CHUNK = cfg["CHUNK"]
    NL = N // NC                      # real nodes per core
    NLP = _ceil_to(NL, P)             # padded nodes per core
    TPC = NLP // P                    # dst tiles per core
    NP = NLP * NC                     # padded global nodes
    INP = _ceil_to(IN, P)             # padded input feature dim
    KC = INP // P                     # k chunks for x @ W1
    LOW = min(32768, NP)              # lo window rows

    # ---- graph WITHOUT self loops (diag handled on-device) ----
    src = edge_index[0].astype(np.int64)
    dst = edge_index[1].astype(np.int64)
    deg = np.bincount(dst, minlength=N).astype(np.float32) + 1.0  # + self loop
    dis = (1.0 / np.sqrt(deg)).astype(np.float32)

    core = dst // NL                  # owning core of dst
    dloc = dst % NL                   # local dst id
    t_of = dloc // P                  # dst tile within core
    dint = (dloc % P).astype(np.float32)  # dst id within tile
    srcp = (src // NL) * NLP + (src % NL)  # padded global src id
    half = (srcp >= LOW).astype(np.int64)  # 0 = lo window, 1 = hi window

    # ---- group counts and uniform tile counts ----
    gid = (core * TPC + t_of) * 2 + half
    cnt = np.bincount(gid, minlength=NC * TPC * 2).reshape(NC, TPC, 2)
    tiles_th = np.ceil(cnt.max(axis=0) / P).astype(np.int64)  # [TPC, 2]

    # ---- chunk layout: for chunk -> for half -> for t in chunk ----
    nchunks = math.ceil(TPC / CHUNK)
    chunk_tiles = [list(range(c * CHUNK, min((c + 1) * CHUNK, TPC))) for c in range(nchunks)]
    base_tile = np.zeros((TPC, 2), np.int64)  # edge-tile offset of group (t, half)
    gathers = []  # per chunk: dict(half -> (base_tile, n_tiles))
    mm_order = []  # per dst tile t: list of global edge-tile indices (lo tiles then hi)
    pos = 0
    tile_pos_of_t = [[[], []] for _ in range(TPC)]
    for tlist in chunk_tiles:
        ginfo = {}
        for h in (0, 1):
            b = pos
            for t in tlist:
                base_tile[t, h] = pos
                tile_pos_of_t[t][h] = list(range(pos, pos + tiles_th[t, h]))
                pos += tiles_th[t, h]
            ginfo[h] = (b, pos - b)
        gathers.append(ginfo)
    total_tiles = pos
    GP = total_tiles * P
    for t in range(TPC):
        mm_order.append(tile_pos_of_t[t][0] + tile_pos_of_t[t][1])

    # ---- per-core padded edge arrays (sorted by src within groups) ----
    order = np.lexsort((srcp, half, t_of, core))
    gid_sorted = gid[order]
    first = np.ones(len(order), bool)
    first[1:] = gid_sorted[1:] != gid_sorted[:-1]
    group_start = np.where(first)[0]
    start_of = np.zeros(NC * TPC * 2, np.int64)
    start_of[gid_sorted[group_start]] = group_start
    rank = np.arange(len(order)) - start_of[gid_sorted]
    slot = base_tile[t_of[order], half[order]] * P + rank

    idx16 = np.zeros((NC, GP), np.int16)
    dstloc = np.full((NC, GP), -1.0, np.float32)
    c_sorted = core[order]
    win_idx = srcp[order] - half[order] * LOW
    assert win_idx.max() < 32768
    idx16[c_sorted, slot] = win_idx.astype(np.int16)
    dstloc[c_sorted, slot] = dint[order]

    # ---- per-core dense inputs ----
    x = np.asarray(x, np.float32)
    W1p = np.zeros((INP, H), np.float32)
    W1p[:IN] = np.asarray(W1, np.float32)
    iota = np.tile(np.arange(P, dtype=np.float32), (P, 1))
    ident = np.eye(P, dtype=np.float32)
    b2rep = np.ascontiguousarray(
        np.tile(np.asarray(b2, np.float32).reshape(1, OUT), (P, 1))
    )

    in_maps = []
    for c in range(NC):
        rows = slice(c * NL, (c + 1) * NL)
        xT = np.zeros((INP, NLP), np.float32)
        xT[:IN, :NL] = x[rows].T
        disl = np.zeros(NLP, np.float32)
        disl[:NL] = dis[rows]
        in_maps.append(
            {
                "xT": xT.astype(BF16),
                "w1": W1p.astype(BF16),
                "w2": np.asarray(W2, np.float32).astype(BF16),
                "b1": np.asarray(b1, np.float32).reshape(H, 1).copy(),
                "b2rep": b2rep,
                "iota": iota.astype(BF16),
                "ident": ident.astype(BF16),
                "dis_local": _wrap128(disl),
                "dis_rep": np.ascontiguousarray(np.tile(disl.reshape(1, NLP), (P, 1))),
                "idx": _wrap16(idx16[c]),
                "dstloc": _wrap128(dstloc[c]).astype(BF16),
            }
        )

    plan = dict(
        cfg=cfg, NL=NL, NLP=NLP, TPC=TPC, NP=NP, INP=INP, KC=KC, LOW=LOW,
        GP=GP, total_tiles=total_tiles, gathers=gathers, mm_order=mm_order,
        chunk_tiles=chunk_tiles,
    )
    return plan, in_maps


# --------------------------------------------------------------------------
# Device program
# --------------------------------------------------------------------------


def _build_program(plan):
    cfg = plan["cfg"]
    N, IN, H, OUT, NC = cfg["N"], cfg["IN"], cfg["H"], cfg["OUT"], cfg["NC"]
    NLP, TPC, NP, INP, KC, LOW, GP = (
        plan["NLP"], plan["TPC"], plan["NP"], plan["INP"], plan["KC"],
        plan["LOW"], plan["GP"],
    )
    gathers, mm_order = plan["gathers"], plan["mm_order"]
    f32 = mybir.dt.float32
    bf16 = mybir.dt.bfloat16

    nc = bacc.Bacc("TRN2", target_bir_lowering=False, debug=False, num_swdge_queues=4)

    xT_d = nc.dram_tensor("xT", [INP, NLP], bf16, kind="ExternalInput")
    w1_d = nc.dram_tensor("w1", [INP, H], bf16, kind="ExternalInput")
    w2_d = nc.dram_tensor("w2", [H, OUT], bf16, kind="ExternalInput")
    b1_d = nc.dram_tensor("b1", [H, 1], f32, kind="ExternalInput")
    b2rep_d = nc.dram_tensor("b2rep", [P, OUT], f32, kind="ExternalInput")
    iota_d = nc.dram_tensor("iota", [P, P], bf16, kind="ExternalInput")
    ident_d = nc.dram_tensor("ident", [P, P], bf16, kind="ExternalInput")
    disl_d = nc.dram_tensor("dis_local", [P, TPC], f32, kind="ExternalInput")
    disrep_d = nc.dram_tensor("dis_rep", [P, NLP], f32, kind="ExternalInput")
    idx_d = nc.dram_tensor("idx", [P, GP // 16], mybir.dt.int16, kind="ExternalInput")
    dstloc_d = nc.dram_tensor("dstloc", [P, GP // P], bf16, kind="ExternalInput")

    p1_local = nc.dram_tensor("p1_local", [NLP, H], bf16)
    p1_full = nc.dram_tensor("p1_full", [NP, H], bf16, addr_space="Shared")
    p2_local = nc.dram_tensor("p2_local", [NLP, P], bf16)
    p2_full = nc.dram_tensor("p2_full", [NP, P], bf16, addr_space="Shared")
    out_d = nc.dram_tensor("out_local", [NLP, OUT], f32, kind="ExternalOutput")

    with tile.TileContext(nc) as tc:
        with (
            tc.tile_pool(name="const", bufs=1) as const_pool,
            tc.tile_pool(name="stage", bufs=1) as stage_pool,
        ):
            # ---- persistent SBUF tiles --------------------------------
            iota_sb = const_pool.tile([P, P], bf16)
            nc.sync.dma_start(iota_sb[:], iota_d[:])
            ident_sb = const_pool.tile([P, P], bf16)
            nc.sync.dma_start(ident_sb[:], ident_d[:])
            b1_sb = const_pool.tile([H, 1], f32)
            nc.sync.dma_start(b1_sb[:], b1_d[:])
            b2rep_sb = const_pool.tile([P, OUT], f32)
            nc.sync.dma_start(b2rep_sb[:], b2rep_d[:])
            disl_sb = const_pool.tile([P, TPC], f32)
            nc.sync.dma_start(disl_sb[:], disl_d[:])
            w2_sb = const_pool.tile([H, OUT], bf16)
            nc.sync.dma_start(w2_sb[:], w2_d[:])
            idx_sb = const_pool.tile([P, GP // 16], mybir.dt.int16)
            nc.sync.dma_start(idx_sb[:], idx_d[:])
            dstloc_sb = const_pool.tile([P, GP // P], bf16)
            nc.sync.dma_start(dstloc_sb[:], dstloc_d[:])
            disrep_sb = const_pool.tile([P, NLP], f32)
            nc.sync.dma_start(disrep_sb[:], disrep_d[:])

            p1_stage = stage_pool.tile([P, TPC, H], bf16)
            p2_stage = stage_pool.tile([P, TPC, OUT], bf16)
            h1T = stage_pool.tile([H, TPC, P], bf16)
            out_stage = stage_pool.tile([P, TPC, OUT], f32)

            # ================= Phase A: p1 = dis * (x @ W1) =============
            with (
                tc.tile_pool(name="xa", bufs=1) as xa_pool,
                tc.tile_pool(name="pa", bufs=4, space="PSUM") as pa_psum,
            ):
                w1_sb = xa_pool.tile([P, KC, H], bf16)
                nc.sync.dma_start(w1_sb[:], w1_d.rearrange("(k p) h -> p k h", p=P))
                xk = xa_pool.tile([P, KC, NLP], bf16)
                nc.sync.dma_start(xk[:], xT_d.rearrange("(k p) n -> p k n", p=P))

                for t in range(TPC):
                    ps = pa_psum.tile([P, H], f32, space="PSUM")
                    for k in range(KC):
                        nc.tensor.matmul(
                            out=ps[:],
                            lhsT=xk[:, k, t * P : (t + 1) * P],
                            rhs=w1_sb[:, k, :],
                            start=(k == 0),
                            stop=(k == KC - 1),
                        )
                    nc.scalar.activation(
                        out=p1_stage[:, t, :],
                        in_=ps[:],
                        func=mybir.ActivationFunctionType.Copy,
                        bias=0.0,
                        scale=disl_sb[:, t : t + 1],
                    )
                nc.sync.dma_start(
                    p1_local.rearrange("(t p) h -> p t h", p=P), p1_stage[:]
                )
                nc.gpsimd.collective_compute(
                    "AllGather",
                    mybir.AluOpType.bypass,
                    replica_groups=[list(range(NC))],
                    ins=[p1_local[:]],
                    outs=[p1_full[:]],
                )

            # ================= Propagate (both layers) ==================
            max_chunk_tiles = max(g[0][1] + g[1][1] for g in gathers)
            with (
                tc.tile_pool(name="msg", bufs=2) as msg_pool,
                tc.tile_pool(name="s", bufs=5) as s_pool,
                tc.tile_pool(name="pb", bufs=2, space="PSUM") as pb_psum,
            ):
                BS = 16  # edge tiles per one-hot build block
                qctr = [0]

                def propagate(table_d, diag_cb, evac_cb):
                    for ci, tlist in enumerate(plan["chunk_tiles"]):
                        g = gathers[ci]
                        nt_lo, nt_hi = g[0][1], g[1][1]
                        ntot = nt_lo + nt_hi
                        if ntot == 0:
                            continue
                        msg = msg_pool.tile([P, max_chunk_tiles, H], bf16, tag="msg")
                        cbase = g[0][0]  # first edge-tile of this chunk
                        for h, off in ((0, 0), (1, nt_lo)):
                            nt = g[h][1]
                            if not nt:
                                continue
                            nidx = nt * P
                            lohi = (0, LOW) if h == 0 else (LOW, NP)
                            nc.gpsimd.dma_gather(
                                msg[:, off : off + nt, :],
                                table_d[lohi[0] : lohi[1], :],
                                idx_sb[:, g[h][0] * 8 : (g[h][0] + nt) * 8],
                                nidx,
                                nidx,
                                H,
                                single_packet=False,
                                queue_num=qctr[0] % 4,
                            )
                            qctr[0] += 1

                        # blocked one-hot builds: S[e, d] = (dstloc_e == d)
                        s_blocks = []
                        for b0 in range(0, ntot, BS):
                            bn = min(BS, ntot - b0)
                            s_blk = s_pool.tile([P, BS * P], bf16, tag="s")
                            dst_b = (
                                dstloc_sb[:, cbase + b0 : cbase + b0 + bn]
                                .unsqueeze(2)
                                .broadcast_to([P, bn, P])
                            )
                            io_b = iota_sb[:].unsqueeze(1).broadcast_to([P, bn, P])
                            nc.vector.tensor_tensor(
                                out=s_blk[:, : bn * P].rearrange(
                                    "p (m f) -> p m f", m=bn
                                ),
                                in0=io_b,
                                in1=dst_b,
                                op=mybir.AluOpType.is_equal,
                            )
                            s_blocks.append(s_blk)

                        for t in tlist:
                            tiles = mm_order[t]
                            ps = pb_psum.tile([P, P], f32, space="PSUM", tag="ps")
                            for j, gt in enumerate(tiles):
                                k = gt - cbase
                                s_t = s_blocks[k // BS][
                                    :, (k % BS) * P : (k % BS + 1) * P
                                ]
                                m = msg[:, k, :]
                                diag_cb(t, ps, s_t, m, j == 0, False)
                            diag_cb(t, ps, None, None, not tiles, True)
                            evac_cb(t, ps)

                # ---- L1: psum[f, d] += msg^T(e,f) x S(e,d); diag: p1[d]
                def l1_mm(t, ps, s_t, m, first, last):
                    if s_t is None:
                        nc.tensor.matmul(
                            out=ps[:, :], lhsT=p1_stage[:, t, :], rhs=ident_sb[:],
                            start=first, stop=True,
                        )
                    else:
                        nc.tensor.matmul(
                            out=ps[:, :], lhsT=m, rhs=s_t, start=first, stop=False
                        )

                def l1_evac(t, ps):
                    tmp = s_pool.tile([P, P], f32, tag="ev1")
                    nc.vector.tensor_tensor(
                        out=tmp[:],
                        in0=ps[:, :],
                        in1=disrep_sb[:, t * P : (t + 1) * P],
                        op=mybir.AluOpType.mult,
                    )
                    nc.scalar.activation(
                        out=h1T[:, t, :],
                        in_=tmp[:],
                        func=mybir.ActivationFunctionType.Relu,
                        bias=b1_sb[:],
                        scale=1.0,
                    )

                propagate(p1_full, l1_mm, l1_evac)

                # ---- L2 transform: p2[d, OUT] = h1T(f,d)^T @ W2(f,OUT)
                for t in range(TPC):
                    ps = pb_psum.tile([P, OUT], f32, space="PSUM", tag="p2p")
                    nc.tensor.matmul(
                        out=ps[:], lhsT=h1T[:, t, :], rhs=w2_sb[:],
                        start=True, stop=True,
                    )
                    nc.scalar.activation(
                        out=p2_stage[:, t, :],
                        in_=ps[:],
                        func=mybir.ActivationFunctionType.Copy,
                        bias=0.0,
                        scale=disl_sb[:, t : t + 1],
                    )
                nc.sync.dma_start(
                    p2_local.rearrange("(t p) h -> p t h", p=P)[:, :, :OUT],
                    p2_stage[:],
                )
                nc.gpsimd.collective_compute(
                    "AllGather",
                    mybir.AluOpType.bypass,
                    replica_groups=[list(range(NC))],
                    ins=[p2_local[:]],
                    outs=[p2_full[:]],
                )

                # ---- L2: psum[d, OUT] += S(e,d)^T @ msg2(e,OUT); diag: p2[d]
                def l2_mm(t, ps, s_t, m, first, last):
                    if s_t is None:
                        nc.tensor.matmul(
                            out=ps[:, :OUT], lhsT=ident_sb[:], rhs=p2_stage[:, t, :],
                            start=first, stop=True,
                        )
                    else:
                        nc.tensor.matmul(
                            out=ps[:, :OUT], lhsT=s_t, rhs=m[:, :OUT],
                            start=first, stop=False,
                        )

                def l2_evac(t, ps):
                    tmp = s_pool.tile([P, OUT], f32, tag="ev2")
                    nc.vector.tensor_scalar(
                        out=tmp[:],
                        in0=ps[:, :OUT],
                        scalar1=disl_sb[:, t : t + 1],
                        scalar2=None,
                        op0=mybir.AluOpType.mult,
                    )
                    nc.vector.tensor_tensor(
                        out=out_stage[:, t, :],
                        in0=tmp[:],
                        in1=b2rep_sb[:],
                        op=mybir.AluOpType.add,
                    )

                propagate(p2_full, l2_mm, l2_evac)

                nc.sync.dma_start(
                    out_d.rearrange("(t p) h -> p t h", p=P), out_stage[:]
                )

    nc.compile()
    return nc


# --------------------------------------------------------------------------
# Entry point
# --------------------------------------------------------------------------


def _run(inputs, cfg=None, trace=False):
    global LAST_RESULTS
    cfg = dict(FULL_CFG if cfg is None else cfg)
    plan, in_maps = _plan_and_prep(
        inputs["x"], inputs["edge_index"], inputs["W1"], inputs["b1"],
        inputs["W2"], inputs["b2"], cfg,
    )
    nc = _build_program(plan)
    if trace:
        _install_ntff_hook()
    res = bass_utils.run_bass_kernel_spmd(
        nc, in_maps, core_ids=list(range(cfg["NC"])), trace=trace
    )
    LAST_RESULTS = res
    NL = plan["NL"]
    out = np.concatenate(
        [res.results[c]["out_local"][:NL] for c in range(cfg["NC"])], axis=0
    )
    return out.astype(np.float32)


def kernel(**inputs):
    return _run(inputs, trace=bool(os.environ.get("GCN_TRACE")))


# revision 13
# speedup vs baseline: 1.5986x; 1.0759x over previous
"""Trainium2 Bass kernel for a 2-layer GCN encoder (PyG GCNConv semantics).

  out = A_hat @ (relu(A_hat @ (x @ W1) + b1) @ W2) + b2
  A_hat = D^-1/2 (A + I) D^-1/2,  deg computed on dst (col) with self loops.

Strategy (8 NeuronCores, SPMD, node sharding), v4:
  - Self-loop edges are not materialized: the diagonal term is added per dst
    tile with one identity matmul against the locally available p1/p2 tile.
  - All tables/matmul operands bf16 (f32 PSUM): p1 tables [*,128] bf16, p2
    tables [*,128] bf16 (64 real cols + pad so gather rows stay 256B).
  - Each AllGather is split in two tile-aligned shard halves (3200+3072 rows
    per core). Both gather windows fit int16, AG-a overlaps Phase A's second
    half, and half-a gathers overlap AG-b.
  - Propagate: per 7-dst-tile chunk, one dma_gather per half (big calls
    amortize the ~2us SWDGE fixed cost), one-hot S[e,d]=(dstloc_e==d) in
    bf16 on DVE, PE matmul reduce into PSUM, diag identity matmul, evac.
  - Host does index prep + layout/dtype conversion only.
"""

import math
import os
import sys
import types

import numpy as np
import ml_dtypes

import concourse.bacc as bacc
import concourse.bass as bass
import concourse.mybir as mybir
import concourse.tile as tile
from concourse import bass_utils


def _install_ntff_hook():
    """Bridge the missing antenv.axon_hooks so trace=True works under axon."""
    try:
        if "antenv.axon_hooks" in sys.modules:
            return
        import trn_agent_boot.trn_boot as tb

        hook = tb._ntff_profile_via_ctypes("/opt/axon/libaxon_pjrt.so")
        mod = types.ModuleType("antenv.axon_hooks")
        mod.get_axon_ntff_profile_hook = lambda: hook
        mod.set_axon_ntff_profile_hook = lambda h: None
        sys.modules["antenv.axon_hooks"] = mod
        import antenv

        antenv.axon_hooks = mod
        bass_utils.upload_artifacts = lambda tmpdir: tmpdir
    except Exception:
        pass

P = 128
BF16 = ml_dtypes.bfloat16

FULL_CFG = dict(N=50000, E=800000, IN=500, H=128, OUT=64, NC=8, CHUNK=7)

LAST_RESULTS = None  # test harness reads exec_time_ns from here


# --------------------------------------------------------------------------
# Host-side preprocessing (index manipulation + input layout only)
# --------------------------------------------------------------------------


def _ceil_to(a, m):
    return (a + m - 1) // m * m


def _wrap16(idx):
    """[G] int16 -> [128, G//16]: edge j at partition j%16 slot j//16, x8 replicated."""
    g = idx.shape[0]
    w = idx.reshape(g // 16, 16).T
    return np.ascontiguousarray(np.tile(w, (8, 1)))


def _wrap128(v):
    """[G] -> [128, G//128]: edge j at partition j%128 slot j//128."""
    g = v.shape[0]
    return np.ascontiguousarray(v.reshape(g // P, P).T)


def _plan_and_prep(x, edge_index, W1, b1, W2, b2, cfg):
    N, E, IN, H, OUT, NC = (
        cfg["N"], cfg["E"], cfg["IN"], cfg["H"], cfg["OUT"], cfg["NC"],
    )
    CHUNK = cfg["CHUNK"]
    NL = N // NC                      # real nodes per core
    NLP = _ceil_to(NL, P)             # padded nodes per core
    TPC = NLP // P                    # dst tiles per core
    NP = NLP * NC                     # padded global nodes
    INP = _ceil_to(IN, P)             # padded input feature dim
    KC = INP // P                     # k chunks for x @ W1
    # split each shard into tile-aligned halves a/b so both gather windows
    # fit int16 and the two AllGathers pipeline
    TA = (TPC + 1) // 2               # a-half tiles per core
    TB = TPC - TA
    SA, SB = TA * P, TB * P           # a/b rows per core
    NPA, NPB = SA * NC, SB * NC       # table rows
    assert NPA < 32768 and NPB < 32768

    # ---- graph WITHOUT self loops (diag handled on-device) ----
    src = edge_index[0].astype(np.int64)
    dst = edge_index[1].astype(np.int64)
    deg = np.bincount(dst, minlength=N).astype(np.float32) + 1.0  # + self loop
    dis = (1.0 / np.sqrt(deg)).astype(np.float32)

    core = dst // NL                  # owning core of dst
    dloc = dst % NL                   # local dst id
    t_of = dloc // P                  # dst tile within core
    dint = (dloc % P).astype(np.float32)  # dst id within tile
    s_core = src // NL                # owning core of src
    s_loc = src % NL                  # local src id
    half = (s_loc >= SA).astype(np.int64)  # 0 = a table, 1 = b table
    win_idx = np.where(half == 0, s_core * SA + s_loc, s_core * SB + (s_loc - SA))
    assert win_idx.max() < 32768

    # ---- group counts and uniform tile counts ----
    gid = (core * TPC + t_of) * 2 + half
    cnt = np.bincount(gid, minlength=NC * TPC * 2).reshape(NC, TPC, 2)
    tiles_th = np.ceil(cnt.max(axis=0) / P).astype(np.int64)  # [TPC, 2]

    # ---- chunk layout: for chunk -> for half -> for t in chunk ----
    nchunks = math.ceil(TPC / CHUNK)
    chunk_tiles = [list(range(c * CHUNK, min((c + 1) * CHUNK, TPC))) for c in range(nchunks)]
    base_tile = np.zeros((TPC, 2), np.int64)  # edge-tile offset of group (t, half)
    gathers = []  # per chunk: dict(half -> (base_tile, n_tiles))
    mm_order = []  # per dst tile t: list of global edge-tile indices (a tiles then b)
    pos = 0
    tile_pos_of_t = [[[], []] for _ in range(TPC)]
    for tlist in chunk_tiles:
        ginfo = {}
        for h in (0, 1):
            b = pos
            for t in tlist:
                base_tile[t, h] = pos
                tile_pos_of_t[t][h] = list(range(pos, pos + tiles_th[t, h]))
                pos += tiles_th[t, h]
            ginfo[h] = (b, pos - b)
        gathers.append(ginfo)
    total_tiles = pos
    GP = total_tiles * P
    for t in range(TPC):
        mm_order.append(tile_pos_of_t[t][0] + tile_pos_of_t[t][1])

    # ---- per-core padded edge arrays (sorted by src within groups) ----
    order = np.lexsort((win_idx, half, t_of, core))
    gid_sorted = gid[order]
    first = np.ones(len(order), bool)
    first[1:] = gid_sorted[1:] != gid_sorted[:-1]
    group_start = np.where(first)[0]
    start_of = np.zeros(NC * TPC * 2, np.int64)
    start_of[gid_sorted[group_start]] = group_start
    rank = np.arange(len(order)) - start_of[gid_sorted]
    slot = base_tile[t_of[order], half[order]] * P + rank

    idx16 = np.zeros((NC, GP), np.int16)
    dstloc = np.full((NC, GP), -1.0, np.float32)
    c_sorted = core[order]
    idx16[c_sorted, slot] = win_idx[order].astype(np.int16)
    dstloc[c_sorted, slot] = dint[order]

    # ---- per-core dense inputs ----
    x = np.asarray(x, np.float32)
    W1p = np.zeros((INP, H), np.float32)
    W1p[:IN] = np.asarray(W1, np.float32)
    iota = np.tile(np.arange(P, dtype=np.float32), (P, 1))
    ident = np.eye(P, dtype=np.float32)
    b2rep = np.ascontiguousarray(
        np.tile(np.asarray(b2, np.float32).reshape(1, OUT), (P, 1))
    )

    in_maps = []
    for c in range(NC):
        rows = slice(c * NL, (c + 1) * NL)
        xT = np.zeros((INP, NLP), np.float32)
        xT[:IN, :NL] = x[rows].T
        disl = np.zeros(NLP, np.float32)
        disl[:NL] = dis[rows]
        in_maps.append(
            {
                "xT": xT.astype(BF16),
                "w1": W1p.astype(BF16),
                "w2": np.asarray(W2, np.float32).astype(BF16),
                "b1": np.asarray(b1, np.float32).reshape(H, 1).copy(),
                "b2rep": b2rep,
                "iota": iota.astype(BF16),
                "ident": ident.astype(BF16),
                "dis_local": _wrap128(disl),
                "dis_rep": np.ascontiguousarray(np.tile(disl.reshape(1, NLP), (P, 1))),
                "idx": _wrap16(idx16[c]),
                "dstloc": _wrap128(dstloc[c]).astype(BF16),
            }
        )

    plan = dict(
        cfg=cfg, NL=NL, NLP=NLP, TPC=TPC, NP=NP, INP=INP, KC=KC,
        TA=TA, TB=TB, SA=SA, SB=SB, NPA=NPA, NPB=NPB,
        GP=GP, total_tiles=total_tiles, gathers=gathers, mm_order=mm_order,
        chunk_tiles=chunk_tiles,
    )
    return plan, in_maps


# --------------------------------------------------------------------------
# Device program
# --------------------------------------------------------------------------


def _build_program(plan):
    cfg = plan["cfg"]
    N, IN, H, OUT, NC = cfg["N"], cfg["IN"], cfg["H"], cfg["OUT"], cfg["NC"]
    NLP, TPC, NP, INP, KC, GP = (
        plan["NLP"], plan["TPC"], plan["NP"], plan["INP"], plan["KC"], plan["GP"],
    )
    TA, TB, SA, SB, NPA, NPB = (
        plan["TA"], plan["TB"], plan["SA"], plan["SB"], plan["NPA"], plan["NPB"],
    )
    gathers, mm_order = plan["gathers"], plan["mm_order"]
    f32 = mybir.dt.float32
    bf16 = mybir.dt.bfloat16

    nc = bacc.Bacc("TRN2", target_bir_lowering=False, debug=False, num_swdge_queues=4)

    xT_d = nc.dram_tensor("xT", [INP, NLP], bf16, kind="ExternalInput")
    w1_d = nc.dram_tensor("w1", [INP, H], bf16, kind="ExternalInput")
    w2_d = nc.dram_tensor("w2", [H, OUT], bf16, kind="ExternalInput")
    b1_d = nc.dram_tensor("b1", [H, 1], f32, kind="ExternalInput")
    b2rep_d = nc.dram_tensor("b2rep", [P, OUT], f32, kind="ExternalInput")
    iota_d = nc.dram_tensor("iota", [P, P], bf16, kind="ExternalInput")
    ident_d = nc.dram_tensor("ident", [P, P], bf16, kind="ExternalInput")
    disl_d = nc.dram_tensor("dis_local", [P, TPC], f32, kind="ExternalInput")
    disrep_d = nc.dram_tensor("dis_rep", [P, NLP], f32, kind="ExternalInput")
    idx_d = nc.dram_tensor("idx", [P, GP // 16], mybir.dt.int16, kind="ExternalInput")
    dstloc_d = nc.dram_tensor("dstloc", [P, GP // P], bf16, kind="ExternalInput")

    p1_loc = [
        nc.dram_tensor("p1_local_a", [SA, H], bf16),
        nc.dram_tensor("p1_local_b", [SB, H], bf16),
    ]
    p1_tab = [
        nc.dram_tensor("p1_tab_a", [NPA, H], bf16, addr_space="Shared"),
        nc.dram_tensor("p1_tab_b", [NPB, H], bf16, addr_space="Shared"),
    ]
    p2_loc = [
        nc.dram_tensor("p2_local_a", [SA, P], bf16),
        nc.dram_tensor("p2_local_b", [SB, P], bf16),
    ]
    p2_tab = [
        nc.dram_tensor("p2_tab_a", [NPA, P], bf16, addr_space="Shared"),
        nc.dram_tensor("p2_tab_b", [NPB, P], bf16, addr_space="Shared"),
    ]
    out_d = nc.dram_tensor("out_local", [NLP, OUT], f32, kind="ExternalOutput")

    with tile.TileContext(nc) as tc:
        with (
            tc.tile_pool(name="const", bufs=1) as const_pool,
            tc.tile_pool(name="stage", bufs=1) as stage_pool,
        ):
            # ---- persistent SBUF tiles --------------------------------
            iota_sb = const_pool.tile([P, P], bf16)
            nc.sync.dma_start(iota_sb[:], iota_d[:])
            ident_sb = const_pool.tile([P, P], bf16)
            nc.sync.dma_start(ident_sb[:], ident_d[:])
            b1_sb = const_pool.tile([H, 1], f32)
            nc.sync.dma_start(b1_sb[:], b1_d[:])
            b2rep_sb = const_pool.tile([P, OUT], f32)
            nc.sync.dma_start(b2rep_sb[:], b2rep_d[:])
            disl_sb = const_pool.tile([P, TPC], f32)
            nc.sync.dma_start(disl_sb[:], disl_d[:])
            w2_sb = const_pool.tile([H, OUT], bf16)
            nc.sync.dma_start(w2_sb[:], w2_d[:])
            idx_sb = const_pool.tile([P, GP // 16], mybir.dt.int16)
            nc.sync.dma_start(idx_sb[:], idx_d[:])
            dstloc_sb = const_pool.tile([P, GP // P], bf16)
            nc.sync.dma_start(dstloc_sb[:], dstloc_d[:])
            disrep_sb = const_pool.tile([P, NLP], f32)
            nc.sync.dma_start(disrep_sb[:], disrep_d[:])

            p1_stage = stage_pool.tile([P, TPC, H], bf16)
            p2_stage = stage_pool.tile([P, TPC, OUT], bf16)
            h1T = stage_pool.tile([H, TPC, P], bf16)

            def allgather(loc, tab):
                nc.gpsimd.collective_compute(
                    "AllGather",
                    mybir.AluOpType.bypass,
                    replica_groups=[list(range(NC))],
                    ins=[loc[:]],
                    outs=[tab[:]],
                )

            # ================= Phase A: p1 = dis * (x @ W1) =============
            with (
                tc.tile_pool(name="xa", bufs=1) as xa_pool,
                tc.tile_pool(name="pa", bufs=4, space="PSUM") as pa_psum,
            ):
                w1_sb = xa_pool.tile([P, KC, H], bf16)
                nc.sync.dma_start(w1_sb[:], w1_d.rearrange("(k p) h -> p k h", p=P))
                xk = xa_pool.tile([P, KC, NLP], bf16)
                nc.sync.dma_start(xk[:], xT_d.rearrange("(k p) n -> p k n", p=P))

                for h, t0, t1, loc in ((0, 0, TA, p1_loc[0]), (1, TA, TPC, p1_loc[1])):
                    for t in range(t0, t1):
                        ps = pa_psum.tile([P, H], f32, space="PSUM")
                        for k in range(KC):
                            nc.tensor.matmul(
                                out=ps[:],
                                lhsT=xk[:, k, t * P : (t + 1) * P],
                                rhs=w1_sb[:, k, :],
                                start=(k == 0),
                                stop=(k == KC - 1),
                            )
                        nc.scalar.activation(
                            out=p1_stage[:, t, :],
                            in_=ps[:],
                            func=mybir.ActivationFunctionType.Copy,
                            bias=0.0,
                            scale=disl_sb[:, t : t + 1],
                        )
                    nc.sync.dma_start(
                        loc.rearrange("(t p) h -> p t h", p=P),
                        p1_stage[:, t0:t1, :],
                    )
                    allgather(loc, p1_tab[h])

            # ================= Propagate (both layers) ==================
            max_chunk_tiles = max(g[0][1] + g[1][1] for g in gathers)
            with (
                tc.tile_pool(name="msg", bufs=2) as msg_pool,
                tc.tile_pool(name="s", bufs=7) as s_pool,
                tc.tile_pool(name="pb", bufs=2, space="PSUM") as pb_psum,
            ):
                BS = 16  # edge tiles per one-hot build block
                qctr = [0]

                def propagate(tabs, diag_cb, evac_cb):
                    for ci, tlist in enumerate(plan["chunk_tiles"]):
                        g = gathers[ci]
                        nt_a, nt_b = g[0][1], g[1][1]
                        ntot = nt_a + nt_b
                        if ntot == 0:
                            continue
                        msg = msg_pool.tile([P, max_chunk_tiles, H], bf16, tag="msg")
                        cbase = g[0][0]  # first edge-tile of this chunk
                        for h, off in ((0, 0), (1, nt_a)):
                            nt = g[h][1]
                            if not nt:
                                continue
                            nidx = nt * P
                            nc.gpsimd.dma_gather(
                                msg[:, off : off + nt, :],
                                tabs[h][:],
                                idx_sb[:, g[h][0] * 8 : (g[h][0] + nt) * 8],
                                nidx,
                                nidx,
                                H,
                                single_packet=False,
                                queue_num=qctr[0] % 4,
                            )
                            qctr[0] += 1

                        # blocked one-hot builds: S[e, d] = (dstloc_e == d)
                        s_blocks = []
                        for b0 in range(0, ntot, BS):
                            bn = min(BS, ntot - b0)
                            s_blk = s_pool.tile([P, BS * P], bf16, tag="s")
                            dst_b = (
                                dstloc_sb[:, cbase + b0 : cbase + b0 + bn]
                                .unsqueeze(2)
                                .broadcast_to([P, bn, P])
                            )
                            io_b = iota_sb[:].unsqueeze(1).broadcast_to([P, bn, P])
                            nc.vector.tensor_tensor(
                                out=s_blk[:, : bn * P].rearrange(
                                    "p (m f) -> p m f", m=bn
                                ),
                                in0=io_b,
                                in1=dst_b,
                                op=mybir.AluOpType.is_equal,
                            )
                            s_blocks.append(s_blk)

                        for t in tlist:
                            tiles = mm_order[t]
                            ps = pb_psum.tile([P, P], f32, space="PSUM", tag="ps")
                            for j, gt in enumerate(tiles):
                                k = gt - cbase
                                s_t = s_blocks[k // BS][
                                    :, (k % BS) * P : (k % BS + 1) * P
                                ]
                                m = msg[:, k, :]
                                diag_cb(t, ps, s_t, m, j == 0, False)
                            diag_cb(t, ps, None, None, not tiles, True)
                            evac_cb(t, ps)

                # ---- L1: psum[f, d] += msg^T(e,f) x S(e,d); diag: p1[d]
                def l1_mm(t, ps, s_t, m, first, last):
                    if s_t is None:
                        nc.tensor.matmul(
                            out=ps[:, :], lhsT=p1_stage[:, t, :], rhs=ident_sb[:],
                            start=first, stop=True,
                        )
                    else:
                        nc.tensor.matmul(
                            out=ps[:, :], lhsT=m, rhs=s_t, start=first, stop=False
                        )

                def l1_evac(t, ps):
                    tmp = s_pool.tile([P, P], f32, tag="ev1")
                    nc.vector.tensor_tensor(
                        out=tmp[:],
                        in0=ps[:, :],
                        in1=disrep_sb[:, t * P : (t + 1) * P],
                        op=mybir.AluOpType.mult,
                    )
                    nc.scalar.activation(
                        out=h1T[:, t, :],
                        in_=tmp[:],
                        func=mybir.ActivationFunctionType.Relu,
                        bias=b1_sb[:],
                        scale=1.0,
                    )

                propagate(p1_tab, l1_mm, l1_evac)

                # ---- L2 transform: p2[d, OUT] = h1T(f,d)^T @ W2(f,OUT)
                for h, t0, t1, loc in ((0, 0, TA, p2_loc[0]), (1, TA, TPC, p2_loc[1])):
                    for t in range(t0, t1):
                        ps = pb_psum.tile([P, OUT], f32, space="PSUM", tag="p2p")
                        nc.tensor.matmul(
                            out=ps[:], lhsT=h1T[:, t, :], rhs=w2_sb[:],
                            start=True, stop=True,
                        )
                        nc.scalar.activation(
                            out=p2_stage[:, t, :],
                            in_=ps[:],
                            func=mybir.ActivationFunctionType.Copy,
                            bias=0.0,
                            scale=disl_sb[:, t : t + 1],
                        )
                    nc.sync.dma_start(
                        loc.rearrange("(t p) h -> p t h", p=P)[:, :, :OUT],
                        p2_stage[:, t0:t1, :],
                    )
                    allgather(loc, p2_tab[h])

                # ---- L2: psum[d, OUT] += S(e,d)^T @ msg2(e,OUT); diag: p2[d]
                def l2_mm(t, ps, s_t, m, first, last):
                    if s_t is None:
                        nc.tensor.matmul(
                            out=ps[:, :OUT], lhsT=ident_sb[:], rhs=p2_stage[:, t, :],
                            start=first, stop=True,
                        )
                    else:
                        nc.tensor.matmul(
                            out=ps[:, :OUT], lhsT=s_t, rhs=m[:, :OUT],
                            start=first, stop=False,
                        )

                outT = out_d.rearrange("(t p) h -> p t h", p=P)

                def l2_evac(t, ps):
                    tmp = s_pool.tile([P, OUT], f32, tag="ev2")
                    nc.vector.tensor_scalar(
                        out=tmp[:],
                        in0=ps[:, :OUT],
                        scalar1=disl_sb[:, t : t + 1],
                        scalar2=None,
                        op0=mybir.AluOpType.mult,
                    )
                    out_t = s_pool.tile([P, OUT], f32, tag="outt")
                    nc.vector.tensor_tensor(
                        out=out_t[:],
                        in0=tmp[:],
                        in1=b2rep_sb[:],
                        op=mybir.AluOpType.add,
                    )
                    nc.sync.dma_start(outT[:, t, :], out_t[:])

                propagate(p2_tab, l2_mm, l2_evac)

    nc.compile()
    return nc


# --------------------------------------------------------------------------
# Entry point
# --------------------------------------------------------------------------


def _run(inputs, cfg=None, trace=False):
    global LAST_RESULTS
    cfg = dict(FULL_CFG if cfg is None else cfg)
    plan, in_maps = _plan_and_prep(
        inputs["x"], inputs["edge_index"], inputs["W1"], inputs["b1"],
        inputs["W2"], inputs["b2"], cfg,
    )
    nc = _build_program(plan)
    if trace:
        _install_ntff_hook()
    res = bass_utils.run_bass_kernel_spmd(
        nc, in_maps, core_ids=list(range(cfg["NC"])), trace=trace
    )
    LAST_RESULTS = res
    NL = plan["NL"]
    out = np.concatenate(
        [res.results[c]["out_local"][:NL] for c in range(cfg["NC"])], axis=0
    )
    return out.astype(np.float32)


def kernel(**inputs):
    return _run(inputs, trace=bool(os.environ.get("GCN_TRACE")))


# revision 17
# speedup vs baseline: 1.6885x; 1.0562x over previous
"""Trainium2 Bass kernel for a 2-layer GCN encoder (PyG GCNConv semantics).

  out = A_hat @ (relu(A_hat @ (x @ W1) + b1) @ W2) + b2
  A_hat = D^-1/2 (A + I) D^-1/2,  deg computed on dst (col) with self loops.

Strategy (8 NeuronCores, SPMD, node sharding), v4:
  - Self-loop edges are not materialized: the diagonal term is added per dst
    tile with one identity matmul against the locally available p1/p2 tile.
  - All tables/matmul operands bf16 (f32 PSUM): p1 tables [*,128] bf16, p2
    tables [*,128] bf16 (64 real cols + pad so gather rows stay 256B).
  - Each AllGather is split in two tile-aligned shard halves (3200+3072 rows
    per core). Both gather windows fit int16, AG-a overlaps Phase A's second
    half, and half-a gathers overlap AG-b.
  - Propagate: per 7-dst-tile chunk, one dma_gather per half (big calls
    amortize the ~2us SWDGE fixed cost), one-hot S[e,d]=(dstloc_e==d) in
    bf16 on DVE, PE matmul reduce into PSUM, diag identity matmul, evac.
  - Host does index prep + layout/dtype conversion only.
"""

import math
import os
import sys
import types

import numpy as np
import ml_dtypes

import concourse.bacc as bacc
import concourse.bass as bass
import concourse.mybir as mybir
import concourse.tile as tile
from concourse import bass_utils


def _install_ntff_hook():
    """Bridge the missing antenv.axon_hooks so trace=True works under axon."""
    try:
        if "antenv.axon_hooks" in sys.modules:
            return
        import trn_agent_boot.trn_boot as tb

        hook = tb._ntff_profile_via_ctypes("/opt/axon/libaxon_pjrt.so")
        mod = types.ModuleType("antenv.axon_hooks")
        mod.get_axon_ntff_profile_hook = lambda: hook
        mod.set_axon_ntff_profile_hook = lambda h: None
        sys.modules["antenv.axon_hooks"] = mod
        import antenv

        antenv.axon_hooks = mod
        bass_utils.upload_artifacts = lambda tmpdir: tmpdir
    except Exception:
        pass

P = 128
BF16 = ml_dtypes.bfloat16

FULL_CFG = dict(N=50000, E=800000, IN=500, H=128, OUT=64, NC=8, CHUNK=5)

LAST_RESULTS = None  # test harness reads exec_time_ns from here


# --------------------------------------------------------------------------
# Host-side preprocessing (index manipulation + input layout only)
# --------------------------------------------------------------------------


def _ceil_to(a, m):
    return (a + m - 1) // m * m


def _wrap16(idx):
    """[G] int16 -> [128, G//16]: edge j at partition j%16 slot j//16, x8 replicated."""
    g = idx.shape[0]
    w = idx.reshape(g // 16, 16).T
    return np.ascontiguousarray(np.tile(w, (8, 1)))


def _wrap128(v):
    """[G] -> [128, G//128]: edge j at partition j%128 slot j//128."""
    g = v.shape[0]
    return np.ascontiguousarray(v.reshape(g // P, P).T)


def _plan_and_prep(x, edge_index, W1, b1, W2, b2, cfg):
    N, E, IN, H, OUT, NC = (
        cfg["N"], cfg["E"], cfg["IN"], cfg["H"], cfg["OUT"], cfg["NC"],
    )
    CHUNK = cfg["CHUNK"]
    NL = N // NC                      # real nodes per core
    NLP = _ceil_to(NL, P)             # padded nodes per core
    TPC = NLP // P                    # dst tiles per core
    NP = NLP * NC                     # padded global nodes
    INP = _ceil_to(IN, P)             # padded input feature dim
    KC = INP // P                     # k chunks for x @ W1
    # split each shard into tile-aligned halves a/b so both gather windows
    # fit int16 and the two AllGathers pipeline
    TA = (TPC + 1) // 2               # a-half tiles per core
    TB = TPC - TA
    SA, SB = TA * P, TB * P           # a/b rows per core
    NPA, NPB = SA * NC, SB * NC       # table rows
    assert NPA < 32768 and NPB < 32768

    # ---- graph WITHOUT self loops (diag handled on-device) ----
    src = edge_index[0].astype(np.int64)
    dst = edge_index[1].astype(np.int64)
    deg = np.bincount(dst, minlength=N).astype(np.float32) + 1.0  # + self loop
    dis = (1.0 / np.sqrt(deg)).astype(np.float32)

    core = dst // NL                  # owning core of dst
    dloc = dst % NL                   # local dst id
    t_of = dloc // P                  # dst tile within core
    dint = (dloc % P).astype(np.float32)  # dst id within tile
    s_core = src // NL                # owning core of src
    s_loc = src % NL                  # local src id
    half = (s_loc >= SA).astype(np.int64)  # 0 = a table, 1 = b table
    win_idx = np.where(half == 0, s_core * SA + s_loc, s_core * SB + (s_loc - SA))
    assert win_idx.max() < 32768

    # ---- group counts and uniform tile counts ----
    gid = (core * TPC + t_of) * 2 + half
    cnt = np.bincount(gid, minlength=NC * TPC * 2).reshape(NC, TPC, 2)
    tiles_th = np.ceil(cnt.max(axis=0) / P).astype(np.int64)  # [TPC, 2]

    # ---- chunk layout: for chunk -> for half -> for t in chunk ----
    nchunks = math.ceil(TPC / CHUNK)
    chunk_tiles = [list(range(c * CHUNK, min((c + 1) * CHUNK, TPC))) for c in range(nchunks)]
    base_tile = np.zeros((TPC, 2), np.int64)  # edge-tile offset of group (t, half)
    gathers = []  # per chunk: dict(half -> (base_tile, n_tiles))
    mm_order = []  # per dst tile t: list of global edge-tile indices (a tiles then b)
    pos = 0
    tile_pos_of_t = [[[], []] for _ in range(TPC)]
    for tlist in chunk_tiles:
        ginfo = {}
        for h in (0, 1):
            b = pos
            for t in tlist:
                base_tile[t, h] = pos
                tile_pos_of_t[t][h] = list(range(pos, pos + tiles_th[t, h]))
                pos += tiles_th[t, h]
            ginfo[h] = (b, pos - b)
        gathers.append(ginfo)
    total_tiles = pos
    GP = total_tiles * P
    for t in range(TPC):
        mm_order.append(tile_pos_of_t[t][0] + tile_pos_of_t[t][1])

    # ---- per-core padded edge arrays (sorted by src within groups) ----
    order = np.lexsort((win_idx, half, t_of, core))
    gid_sorted = gid[order]
    first = np.ones(len(order), bool)
    first[1:] = gid_sorted[1:] != gid_sorted[:-1]
    group_start = np.where(first)[0]
    start_of = np.zeros(NC * TPC * 2, np.int64)
    start_of[gid_sorted[group_start]] = group_start
    rank = np.arange(len(order)) - start_of[gid_sorted]
    slot = base_tile[t_of[order], half[order]] * P + rank

    idx16 = np.zeros((NC, GP), np.int16)
    c_sorted = core[order]
    idx16[c_sorted, slot] = win_idx[order].astype(np.int16)
    # host-built one-hot scatter matrices: S[tile, e, d] = (dst-in-tile == d)
    # stored partition-major: S_dram[c][e, tile*128 + d]
    s_dram = np.zeros((NC, P, total_tiles * P), BF16)
    tile_of_slot = slot // P
    e_of_slot = slot % P
    s_dram[c_sorted, e_of_slot, tile_of_slot * P + dint[order].astype(np.int64)] = 1.0

    # ---- per-core dense inputs ----
    x = np.asarray(x, np.float32)
    W1p = np.zeros((INP, H), np.float32)
    W1p[:IN] = np.asarray(W1, np.float32)
    ident = np.eye(P, dtype=np.float32)
    b2rep = np.ascontiguousarray(
        np.tile(np.asarray(b2, np.float32).reshape(1, OUT), (P, 1))
    )

    in_maps = []
    for c in range(NC):
        rows = slice(c * NL, (c + 1) * NL)
        xT = np.zeros((INP, NLP), np.float32)
        xT[:IN, :NL] = x[rows].T
        disl = np.zeros(NLP, np.float32)
        disl[:NL] = dis[rows]
        in_maps.append(
            {
                "xT": xT.astype(BF16),
                "w1": W1p.astype(BF16),
                "w2": np.asarray(W2, np.float32).astype(BF16),
                "b1": np.asarray(b1, np.float32).reshape(H, 1).copy(),
                "b2rep": b2rep,
                "ident": ident.astype(BF16),
                "dis_local": _wrap128(disl),
                "dis_rep": np.ascontiguousarray(np.tile(disl.reshape(1, NLP), (P, 1))),
                "idx": _wrap16(idx16[c]),
                "s_mat": np.ascontiguousarray(s_dram[c]),
            }
        )

    plan = dict(
        cfg=cfg, NL=NL, NLP=NLP, TPC=TPC, NP=NP, INP=INP, KC=KC,
        TA=TA, TB=TB, SA=SA, SB=SB, NPA=NPA, NPB=NPB,
        GP=GP, total_tiles=total_tiles, gathers=gathers, mm_order=mm_order,
        chunk_tiles=chunk_tiles,
    )
    return plan, in_maps


# --------------------------------------------------------------------------
# Device program
# --------------------------------------------------------------------------


def _build_program(plan):
    cfg = plan["cfg"]
    N, IN, H, OUT, NC = cfg["N"], cfg["IN"], cfg["H"], cfg["OUT"], cfg["NC"]
    NLP, TPC, NP, INP, KC, GP = (
        plan["NLP"], plan["TPC"], plan["NP"], plan["INP"], plan["KC"], plan["GP"],
    )
    TA, TB, SA, SB, NPA, NPB = (
        plan["TA"], plan["TB"], plan["SA"], plan["SB"], plan["NPA"], plan["NPB"],
    )
    gathers, mm_order = plan["gathers"], plan["mm_order"]
    f32 = mybir.dt.float32
    bf16 = mybir.dt.bfloat16

    nc = bacc.Bacc("TRN2", target_bir_lowering=False, debug=False, num_swdge_queues=4)

    xT_d = nc.dram_tensor("xT", [INP, NLP], bf16, kind="ExternalInput")
    w1_d = nc.dram_tensor("w1", [INP, H], bf16, kind="ExternalInput")
    w2_d = nc.dram_tensor("w2", [H, OUT], bf16, kind="ExternalInput")
    b1_d = nc.dram_tensor("b1", [H, 1], f32, kind="ExternalInput")
    b2rep_d = nc.dram_tensor("b2rep", [P, OUT], f32, kind="ExternalInput")
    ident_d = nc.dram_tensor("ident", [P, P], bf16, kind="ExternalInput")
    disl_d = nc.dram_tensor("dis_local", [P, TPC], f32, kind="ExternalInput")
    disrep_d = nc.dram_tensor("dis_rep", [P, NLP], f32, kind="ExternalInput")
    idx_d = nc.dram_tensor("idx", [P, GP // 16], mybir.dt.int16, kind="ExternalInput")
    smat_d = nc.dram_tensor(
        "s_mat", [P, plan["total_tiles"] * P], bf16, kind="ExternalInput"
    )

    p1_loc = [
        nc.dram_tensor("p1_local_a", [SA, H], bf16),
        nc.dram_tensor("p1_local_b", [SB, H], bf16),
    ]
    p1_tab = [
        nc.dram_tensor("p1_tab_a", [NPA, H], bf16, addr_space="Shared"),
        nc.dram_tensor("p1_tab_b", [NPB, H], bf16, addr_space="Shared"),
    ]
    p2_loc = [
        nc.dram_tensor("p2_local_a", [SA, P], bf16),
        nc.dram_tensor("p2_local_b", [SB, P], bf16),
    ]
    p2_tab = [
        nc.dram_tensor("p2_tab_a", [NPA, P], bf16, addr_space="Shared"),
        nc.dram_tensor("p2_tab_b", [NPB, P], bf16, addr_space="Shared"),
    ]
    out_d = nc.dram_tensor("out_local", [NLP, OUT], f32, kind="ExternalOutput")

    with tile.TileContext(nc) as tc:
        with (
            tc.tile_pool(name="const", bufs=1) as const_pool,
            tc.tile_pool(name="stage", bufs=1) as stage_pool,
        ):
            # ---- persistent SBUF tiles --------------------------------
            ident_sb = const_pool.tile([P, P], bf16)
            nc.sync.dma_start(ident_sb[:], ident_d[:])
            b1_sb = const_pool.tile([H, 1], f32)
            nc.sync.dma_start(b1_sb[:], b1_d[:])
            b2rep_sb = const_pool.tile([P, OUT], f32)
            nc.sync.dma_start(b2rep_sb[:], b2rep_d[:])
            disl_sb = const_pool.tile([P, TPC], f32)
            nc.sync.dma_start(disl_sb[:], disl_d[:])
            w2_sb = const_pool.tile([H, OUT], bf16)
            nc.sync.dma_start(w2_sb[:], w2_d[:])
            idx_sb = const_pool.tile([P, GP // 16], mybir.dt.int16)
            nc.sync.dma_start(idx_sb[:], idx_d[:])
            disrep_sb = const_pool.tile([P, NLP], f32)
            nc.sync.dma_start(disrep_sb[:], disrep_d[:])

            p1_stage = stage_pool.tile([P, TPC, H], bf16)
            p2_stage = stage_pool.tile([P, TPC, OUT], bf16)
            h1T = stage_pool.tile([H, TPC, P], bf16)

            def allgather(loc, tab):
                nc.gpsimd.collective_compute(
                    "AllGather",
                    mybir.AluOpType.bypass,
                    replica_groups=[list(range(NC))],
                    ins=[loc[:]],
                    outs=[tab[:]],
                )

            # ================= Phase A: p1 = dis * (x @ W1) =============
            with (
                tc.tile_pool(name="xa", bufs=1) as xa_pool,
                tc.tile_pool(name="pa", bufs=4, space="PSUM") as pa_psum,
            ):
                w1_sb = xa_pool.tile([P, KC, H], bf16)
                nc.sync.dma_start(w1_sb[:], w1_d.rearrange("(k p) h -> p k h", p=P))
                xk = xa_pool.tile([P, KC, NLP], bf16)
                nc.sync.dma_start(xk[:], xT_d.rearrange("(k p) n -> p k n", p=P))

                for h, t0, t1, loc in ((0, 0, TA, p1_loc[0]), (1, TA, TPC, p1_loc[1])):
                    for t in range(t0, t1):
                        ps = pa_psum.tile([P, H], f32, space="PSUM")
                        for k in range(KC):
                            nc.tensor.matmul(
                                out=ps[:],
                                lhsT=xk[:, k, t * P : (t + 1) * P],
                                rhs=w1_sb[:, k, :],
                                start=(k == 0),
                                stop=(k == KC - 1),
                            )
                        nc.scalar.activation(
                            out=p1_stage[:, t, :],
                            in_=ps[:],
                            func=mybir.ActivationFunctionType.Copy,
                            bias=0.0,
                            scale=disl_sb[:, t : t + 1],
                        )
                    nc.sync.dma_start(
                        loc.rearrange("(t p) h -> p t h", p=P),
                        p1_stage[:, t0:t1, :],
                    )
                    allgather(loc, p1_tab[h])

            # ================= Propagate (both layers) ==================
            max_chunk_tiles = max(g[0][1] + g[1][1] for g in gathers)
            with (
                tc.tile_pool(name="msg", bufs=3) as msg_pool,
                tc.tile_pool(name="s", bufs=2) as s_pool,
                tc.tile_pool(name="ev", bufs=4) as ev_pool,
                tc.tile_pool(name="pb", bufs=2, space="PSUM") as pb_psum,
            ):
                BS = 16  # edge tiles per one-hot build block
                qctr = [0]

                def propagate(tabs, diag_cb, evac_cb):
                    for ci, tlist in enumerate(plan["chunk_tiles"]):
                        g = gathers[ci]
                        nt_a, nt_b = g[0][1], g[1][1]
                        ntot = nt_a + nt_b
                        if ntot == 0:
                            continue
                        msg = msg_pool.tile([P, max_chunk_tiles, H], bf16, tag="msg")
                        cbase = g[0][0]  # first edge-tile of this chunk
                        for h, off in ((0, 0), (1, nt_a)):
                            nt = g[h][1]
                            if not nt:
                                continue
                            nidx = nt * P
                            nc.gpsimd.dma_gather(
                                msg[:, off : off + nt, :],
                                tabs[h][:],
                                idx_sb[:, g[h][0] * 8 : (g[h][0] + nt) * 8],
                                nidx,
                                nidx,
                                H,
                                single_packet=False,
                                queue_num=qctr[0] % 4,
                            )
                            qctr[0] += 1

                        # stream host-built one-hot block for this chunk
                        s_chunk = s_pool.tile(
                            [P, max_chunk_tiles * P], bf16, tag="s"
                        )
                        nc.sync.dma_start(
                            s_chunk[:, : ntot * P],
                            smat_d[:, cbase * P : (cbase + ntot) * P],
                        )

                        for t in tlist:
                            tiles = mm_order[t]
                            ps = pb_psum.tile([P, P], f32, space="PSUM", tag="ps")
                            for j, gt in enumerate(tiles):
                                k = gt - cbase
                                s_t = s_chunk[:, k * P : (k + 1) * P]
                                m = msg[:, k, :]
                                diag_cb(t, ps, s_t, m, j == 0, False)
                            diag_cb(t, ps, None, None, not tiles, True)
                            evac_cb(t, ps)

                # ---- L1: psum[f, d] += msg^T(e,f) x S(e,d); diag: p1[d]
                def l1_mm(t, ps, s_t, m, first, last):
                    if s_t is None:
                        nc.tensor.matmul(
                            out=ps[:, :], lhsT=p1_stage[:, t, :], rhs=ident_sb[:],
                            start=first, stop=True,
                        )
                    else:
                        nc.tensor.matmul(
                            out=ps[:, :], lhsT=m, rhs=s_t, start=first, stop=False
                        )

                def l1_evac(t, ps):
                    tmp = ev_pool.tile([P, P], f32, tag="ev1")
                    nc.vector.tensor_tensor(
                        out=tmp[:],
                        in0=ps[:, :],
                        in1=disrep_sb[:, t * P : (t + 1) * P],
                        op=mybir.AluOpType.mult,
                    )
                    nc.scalar.activation(
                        out=h1T[:, t, :],
                        in_=tmp[:],
                        func=mybir.ActivationFunctionType.Relu,
                        bias=b1_sb[:],
                        scale=1.0,
                    )

                propagate(p1_tab, l1_mm, l1_evac)

                # ---- L2 transform: p2[d, OUT] = h1T(f,d)^T @ W2(f,OUT)
                for h, t0, t1, loc in ((0, 0, TA, p2_loc[0]), (1, TA, TPC, p2_loc[1])):
                    for t in range(t0, t1):
                        ps = pb_psum.tile([P, OUT], f32, space="PSUM", tag="p2p")
                        nc.tensor.matmul(
                            out=ps[:], lhsT=h1T[:, t, :], rhs=w2_sb[:],
                            start=True, stop=True,
                        )
                        nc.scalar.activation(
                            out=p2_stage[:, t, :],
                            in_=ps[:],
                            func=mybir.ActivationFunctionType.Copy,
                            bias=0.0,
                            scale=disl_sb[:, t : t + 1],
                        )
                    nc.sync.dma_start(
                        loc.rearrange("(t p) h -> p t h", p=P)[:, :, :OUT],
                        p2_stage[:, t0:t1, :],
                    )
                    allgather(loc, p2_tab[h])

                # ---- L2: psum[d, OUT] += S(e,d)^T @ msg2(e,OUT); diag: p2[d]
                def l2_mm(t, ps, s_t, m, first, last):
                    if s_t is None:
                        nc.tensor.matmul(
                            out=ps[:, :OUT], lhsT=ident_sb[:], rhs=p2_stage[:, t, :],
                            start=first, stop=True,
                        )
                    else:
                        nc.tensor.matmul(
                            out=ps[:, :OUT], lhsT=s_t, rhs=m[:, :OUT],
                            start=first, stop=False,
                        )

                outT = out_d.rearrange("(t p) h -> p t h", p=P)

                def l2_evac(t, ps):
                    tmp = ev_pool.tile([P, OUT], f32, tag="ev2")
                    nc.vector.tensor_scalar(
                        out=tmp[:],
                        in0=ps[:, :OUT],
                        scalar1=disl_sb[:, t : t + 1],
                        scalar2=None,
                        op0=mybir.AluOpType.mult,
                    )
                    out_t = ev_pool.tile([P, OUT], f32, tag="outt")
                    nc.vector.tensor_tensor(
                        out=out_t[:],
                        in0=tmp[:],
                        in1=b2rep_sb[:],
                        op=mybir.AluOpType.add,
                    )
                    nc.sync.dma_start(outT[:, t, :], out_t[:])

                propagate(p2_tab, l2_mm, l2_evac)

    nc.compile()
    return nc


# --------------------------------------------------------------------------
# Entry point
# --------------------------------------------------------------------------


def _run(inputs, cfg=None, trace=False):
    global LAST_RESULTS
    cfg = dict(FULL_CFG if cfg is None else cfg)
    plan, in_maps = _plan_and_prep(
        inputs["x"], inputs["edge_index"], inputs["W1"], inputs["b1"],
        inputs["W2"], inputs["b2"], cfg,
    )
    nc = _build_program(plan)
    if trace:
        _install_ntff_hook()
    res = bass_utils.run_bass_kernel_spmd(
        nc, in_maps, core_ids=list(range(cfg["NC"])), trace=trace
    )
    LAST_RESULTS = res
    NL = plan["NL"]
    out = np.concatenate(
        [res.results[c]["out_local"][:NL] for c in range(cfg["NC"])], axis=0
    )
    return out.astype(np.float32)


def kernel(**inputs):
    return _run(inputs, trace=bool(os.environ.get("GCN_TRACE")))
